# revision 5
# baseline (speedup 1.0000x reference)
"""Trainium2 Bass kernel for nn_Loss_2 (weighted BCE + index-gathered CE mean).

Data-parallel over 8 NeuronCores: each core processes 8 of the 64 batches,
computes per-partition partial sums on-chip, host sums 8x[128,1] partials and
divides by B*S.

Per-core program (tokens laid out [NT, 128, Tp] contiguous):
  LnC  = Ln(comb)                          (ScalarE, bf16)
  idxg = y_comb + (1-ys)*64                (pushes ys==0 tokens out of [0,20))
  D    = idxg_bcast - iota_class           (DVE, bf16; ==0 exactly at gathered class)
  cce_p = sum((D==0) * LnC)                (DVE scalar_tensor_tensor + accum_out)
  b1_p  = sum((ys*-W1) * Ln(ps))           (DVE scalar_tensor_tensor + accum_out)
  b0_p  = sum(((1-ys)*-W0) * Ln(1-ps))     (DVE scalar_tensor_tensor + accum_out)
  acc  += b1_p + b0_p - cce_p
"""

import sys

if '/opt/trn_rl_repo' not in sys.path:
    sys.path.insert(0, '/opt/trn_rl_repo')

import numpy as np

import concourse.bass as bass
import concourse.bacc as bacc
import concourse.tile as tile
import concourse.mybir as mybir
from concourse.bass_utils import run_bass_kernel_spmd

F32 = mybir.dt.float32
BF16 = mybir.dt.bfloat16
I32 = mybir.dt.int32
I16 = mybir.dt.int16

B, S, C = 64, 16384, 20
W0, W1 = 0.51, 19.05
BIG = 64.0
P = 128
N_CORES = 8
Tp = 256                       # tokens per partition per tile
NT = (B // N_CORES) * S // (P * Tp)  # 4 tiles per core


def _build(NT, Tp, comb_bufs=3):
    FREE = Tp * C
    nc = bacc.Bacc("TRN2", target_bir_lowering=False, debug=False)

    comb_d = nc.dram_tensor("comb", [NT, P, FREE], F32, kind="ExternalInput").ap()
    idxg_d = nc.dram_tensor("idxg", [NT, P, Tp], F32, kind="ExternalInput").ap()
    ps_d = nc.dram_tensor("ps", [NT, P, Tp], F32, kind="ExternalInput").ap()
    ys_d = nc.dram_tensor("ys", [NT, P, Tp], F32, kind="ExternalInput").ap()
    iotac_d = nc.dram_tensor("iotac", [P, FREE], I16, kind="ExternalInput").ap()
    out_d = nc.dram_tensor("out", [P, 1], F32, kind="ExternalOutput").ap()

    with tile.TileContext(nc) as tc:
        with (
            tc.tile_pool(name="const", bufs=1) as const_pool,
            tc.tile_pool(name="comb", bufs=comb_bufs) as comb_pool,
            tc.tile_pool(name="big", bufs=3) as big_pool,
            tc.tile_pool(name="small", bufs=3) as small_pool,
        ):
            iota_t = const_pool.tile([P, FREE], I16)
            nc.sync.dma_start(iota_t[:], iotac_d[:])
            iota_v = iota_t[:].rearrange("p (t c) -> p t c", c=C)

            partsA = const_pool.tile([P, 2 * NT], F32)
            partsB = const_pool.tile([P, NT], F32)

            for i in range(NT):
                comb_t = comb_pool.tile([P, FREE], F32, tag="comb")
                nc.sync.dma_start(comb_t[:], comb_d[i])
                idxg = small_pool.tile([P, Tp], F32, tag="idxg")
                nc.sync.dma_start(idxg[:], idxg_d[i])
                ps_t = small_pool.tile([P, Tp], F32, tag="ps")
                nc.sync.dma_start(ps_t[:], ps_d[i])
                ys_t = small_pool.tile([P, Tp], F32, tag="ys")
                nc.sync.dma_start(ys_t[:], ys_d[i])

                lnc = big_pool.tile([P, FREE], BF16, tag="lnc")
                nc.scalar.activation(lnc[:], comb_t[:], mybir.ActivationFunctionType.Ln)

                idxg_b = idxg[:].rearrange("p (t o) -> p t o", o=1)

                mask = big_pool.tile([P, FREE], BF16, tag="mask")
                mask_v = mask[:].rearrange("p (t c) -> p t c", c=C)
                b_iota, b_idxg = bass.broadcast_tensor_aps(iota_v, idxg_b)
                nc.vector.tensor_tensor(mask_v, b_iota, b_idxg,
                                        mybir.AluOpType.is_equal)

                if False:
                    # DVE-only path: fused mult+sum on VectorE
                    nc.vector.scalar_tensor_tensor(
                        mask[:], mask[:], 1.0, lnc[:],
                        op0=mybir.AluOpType.mult, op1=mybir.AluOpType.mult,
                        accum_out=partsB[:, i:i + 1],
                    )
                else:
                    # split path: 2x bf16 multiply on DVE, sum on ScalarE
                    prod = big_pool.tile([P, FREE], BF16, tag="prod")
                    nc.vector.tensor_tensor(prod[:], mask[:], lnc[:],
                                            mybir.AluOpType.mult)
                    nc.scalar.activation(prod[:], prod[:],
                                         mybir.ActivationFunctionType.Copy,
                                         accum_out=partsB[:, i:i + 1])

                lps = small_pool.tile([P, Tp], F32, tag="lps")
                nc.scalar.activation(lps[:], ps_t[:], mybir.ActivationFunctionType.Ln)
                l1m = small_pool.tile([P, Tp], F32, tag="l1m")
                nc.scalar.activation(l1m[:], ps_t[:], mybir.ActivationFunctionType.Ln,
                                     bias=1.0, scale=-1.0)

                nc.vector.scalar_tensor_tensor(
                    lps[:], ys_t[:], -W1, lps[:],
                    op0=mybir.AluOpType.mult, op1=mybir.AluOpType.mult,
                    accum_out=partsA[:, 2 * i:2 * i + 1],
                )

                ys1m = small_pool.tile([P, Tp], F32, tag="ys1m")
                nc.vector.tensor_scalar(ys1m[:], ys_t[:], -1.0, 1.0,
                                        mybir.AluOpType.mult, mybir.AluOpType.add)
                nc.vector.scalar_tensor_tensor(
                    l1m[:], ys1m[:], -W0, l1m[:],
                    op0=mybir.AluOpType.mult, op1=mybir.AluOpType.mult,
                    accum_out=partsA[:, 2 * i + 1:2 * i + 2],
                )

            rA = const_pool.tile([P, 1], F32)
            nc.vector.tensor_reduce(rA[:], partsA[:], axis=mybir.AxisListType.X,
                                    op=mybir.AluOpType.add)
            rB = const_pool.tile([P, 1], F32)
            nc.vector.tensor_reduce(rB[:], partsB[:], axis=mybir.AxisListType.X,
                                    op=mybir.AluOpType.add)
            total = const_pool.tile([P, 1], F32)
            nc.vector.tensor_tensor(total[:], rA[:], rB[:], mybir.AluOpType.subtract)

            nc.sync.dma_start(out_d[:], total[:])

    nc.compile()
    return nc


_NC_CACHE = {}
IOTAC = np.ascontiguousarray(
    np.broadcast_to(np.tile(np.arange(C, dtype=np.int16), Tp), (P, Tp * C)))


def kernel(y_pred_stroke, y_pred_comb, y_stroke, y_comb):
    y_pred_stroke = np.asarray(y_pred_stroke, dtype=np.float32)
    y_pred_comb = np.asarray(y_pred_comb, dtype=np.float32)
    y_stroke = np.asarray(y_stroke, dtype=np.float32)
    y_comb = np.asarray(y_comb)

    key = (NT, Tp)
    if key not in _NC_CACHE:
        _NC_CACHE[key] = _build(NT, Tp)
    nc = _NC_CACHE[key]

    FREE = Tp * C
    Bc = B // N_CORES
    in_maps = []
    for c in range(N_CORES):
        sl = slice(c * Bc, (c + 1) * Bc)
        in_maps.append({
            "comb": np.ascontiguousarray(y_pred_comb[sl]).reshape(NT, P, FREE),
            "idxg": (np.ascontiguousarray(y_comb[sl]).astype(np.float32)
                     + (1.0 - np.ascontiguousarray(y_stroke[sl])[..., 0]) * BIG
                     ).reshape(NT, P, Tp),
            "ps": np.ascontiguousarray(y_pred_stroke[sl]).reshape(NT, P, Tp),
            "ys": np.ascontiguousarray(y_stroke[sl]).reshape(NT, P, Tp),
            "iotac": IOTAC,
        })

    res = run_bass_kernel_spmd(nc, in_maps, list(range(N_CORES)))
    total = 0.0
    for r in res.results:
        total += r["out"].astype(np.float64).sum()
    return np.asarray([total / (B * S)], dtype=np.float32)


# revision 6
# speedup vs baseline: 1.0883x; 1.0883x over previous
"""Trainium2 Bass kernel for nn_Loss_2 (weighted BCE + index-gathered CE mean).

Data-parallel over 8 NeuronCores: each core processes 8 of the 64 batches,
computes per-partition partial sums on-chip, host sums 8x[128,1] partials and
divides by B*S.

Per-core program (tokens laid out [NT, 128, Tp] contiguous):
  LnC  = Ln(comb)                          (ScalarE, bf16)
  idxg = y_comb + (1-ys)*64                (pushes ys==0 tokens out of [0,20))
  D    = idxg_bcast - iota_class           (DVE, bf16; ==0 exactly at gathered class)
  cce_p = sum((D==0) * LnC)                (DVE scalar_tensor_tensor + accum_out)
  b1_p  = sum((ys*-W1) * Ln(ps))           (DVE scalar_tensor_tensor + accum_out)
  b0_p  = sum(((1-ys)*-W0) * Ln(1-ps))     (DVE scalar_tensor_tensor + accum_out)
  acc  += b1_p + b0_p - cce_p
"""

import sys

if '/opt/trn_rl_repo' not in sys.path:
    sys.path.insert(0, '/opt/trn_rl_repo')

import numpy as np

import concourse.bass as bass
import concourse.bacc as bacc
import concourse.tile as tile
import concourse.mybir as mybir
from concourse.bass_utils import run_bass_kernel_spmd

F32 = mybir.dt.float32
BF16 = mybir.dt.bfloat16
I32 = mybir.dt.int32
I16 = mybir.dt.int16

B, S, C = 64, 16384, 20
W0, W1 = 0.51, 19.05
BIG = 64.0
P = 128
N_CORES = 8
Tp = 256                       # tokens per partition per tile
NT = (B // N_CORES) * S // (P * Tp)  # 4 tiles per core


def _build(NT, Tp, comb_bufs=2):
    FREE = Tp * C
    nc = bacc.Bacc("TRN2", target_bir_lowering=False, debug=False)

    comb_d = nc.dram_tensor("comb", [NT, P, FREE], F32, kind="ExternalInput").ap()
    idxg_d = nc.dram_tensor("idxg", [NT, P, Tp], F32, kind="ExternalInput").ap()
    ps_d = nc.dram_tensor("ps", [NT, P, Tp], F32, kind="ExternalInput").ap()
    ys_d = nc.dram_tensor("ys", [NT, P, Tp], F32, kind="ExternalInput").ap()
    out_d = nc.dram_tensor("out", [P, 1], F32, kind="ExternalOutput").ap()

    with tile.TileContext(nc) as tc:
        with (
            tc.tile_pool(name="const", bufs=1) as const_pool,
            tc.tile_pool(name="comb", bufs=comb_bufs) as comb_pool,
            tc.tile_pool(name="big", bufs=2) as big_pool,
            tc.tile_pool(name="small", bufs=3) as small_pool,
        ):
            iota_t = const_pool.tile([P, FREE], I16)
            nc.gpsimd.iota(iota_t[:], pattern=[[0, Tp], [1, C]], base=0,
                           channel_multiplier=0)
            iota_v = iota_t[:].rearrange("p (t c) -> p t c", c=C)

            partsA = const_pool.tile([P, 2 * NT], F32)
            partsB = const_pool.tile([P, NT], F32)

            for i in range(NT):
                comb_t = comb_pool.tile([P, FREE], F32, tag="comb")
                nc.sync.dma_start(comb_t[:], comb_d[i])
                idxg = small_pool.tile([P, Tp], F32, tag="idxg")
                nc.sync.dma_start(idxg[:], idxg_d[i])
                ps_t = small_pool.tile([P, Tp], F32, tag="ps")
                nc.sync.dma_start(ps_t[:], ps_d[i])
                ys_t = small_pool.tile([P, Tp], F32, tag="ys")
                nc.sync.dma_start(ys_t[:], ys_d[i])

                lnc = big_pool.tile([P, FREE], BF16, tag="lnc")
                nc.scalar.activation(lnc[:], comb_t[:], mybir.ActivationFunctionType.Ln)

                idxg_b = idxg[:].rearrange("p (t o) -> p t o", o=1)

                mask = big_pool.tile([P, FREE], BF16, tag="mask")
                mask_v = mask[:].rearrange("p (t c) -> p t c", c=C)
                b_iota, b_idxg = bass.broadcast_tensor_aps(iota_v, idxg_b)
                nc.vector.tensor_tensor(mask_v, b_iota, b_idxg,
                                        mybir.AluOpType.is_equal)

                if False:
                    # DVE-only path: fused mult+sum on VectorE
                    nc.vector.scalar_tensor_tensor(
                        mask[:], mask[:], 1.0, lnc[:],
                        op0=mybir.AluOpType.mult, op1=mybir.AluOpType.mult,
                        accum_out=partsB[:, i:i + 1],
                    )
                else:
                    # split path: 2x bf16 multiply on DVE, sum on ScalarE
                    prod = big_pool.tile([P, FREE], BF16, tag="prod")
                    nc.vector.tensor_tensor(prod[:], mask[:], lnc[:],
                                            mybir.AluOpType.mult)
                    nc.scalar.activation(prod[:], prod[:],
                                         mybir.ActivationFunctionType.Copy,
                                         accum_out=partsB[:, i:i + 1])

                lps = small_pool.tile([P, Tp], F32, tag="lps")
                nc.scalar.activation(lps[:], ps_t[:], mybir.ActivationFunctionType.Ln)
                l1m = small_pool.tile([P, Tp], F32, tag="l1m")
                nc.scalar.activation(l1m[:], ps_t[:], mybir.ActivationFunctionType.Ln,
                                     bias=1.0, scale=-1.0)

                nc.vector.scalar_tensor_tensor(
                    lps[:], ys_t[:], -W1, lps[:],
                    op0=mybir.AluOpType.mult, op1=mybir.AluOpType.mult,
                    accum_out=partsA[:, 2 * i:2 * i + 1],
                )

                ys1m = small_pool.tile([P, Tp], F32, tag="ys1m")
                nc.vector.tensor_scalar(ys1m[:], ys_t[:], -1.0, 1.0,
                                        mybir.AluOpType.mult, mybir.AluOpType.add)
                nc.vector.scalar_tensor_tensor(
                    l1m[:], ys1m[:], -W0, l1m[:],
                    op0=mybir.AluOpType.mult, op1=mybir.AluOpType.mult,
                    accum_out=partsA[:, 2 * i + 1:2 * i + 2],
                )

            rA = const_pool.tile([P, 1], F32)
            nc.vector.tensor_reduce(rA[:], partsA[:], axis=mybir.AxisListType.X,
                                    op=mybir.AluOpType.add)
            rB = const_pool.tile([P, 1], F32)
            nc.vector.tensor_reduce(rB[:], partsB[:], axis=mybir.AxisListType.X,
                                    op=mybir.AluOpType.add)
            total = const_pool.tile([P, 1], F32)
            nc.vector.tensor_tensor(total[:], rA[:], rB[:], mybir.AluOpType.subtract)

            nc.sync.dma_start(out_d[:], total[:])

    nc.compile()
    return nc


_NC_CACHE = {}
IOTAC = np.ascontiguousarray(
    np.broadcast_to(np.tile(np.arange(C, dtype=np.int16), Tp), (P, Tp * C)))


def make_in_maps(y_pred_stroke, y_pred_comb, y_stroke, y_comb):
    y_pred_stroke = np.asarray(y_pred_stroke, dtype=np.float32)
    y_pred_comb = np.asarray(y_pred_comb, dtype=np.float32)
    y_stroke = np.asarray(y_stroke, dtype=np.float32)
    y_comb = np.asarray(y_comb)
    FREE = Tp * C
    Bc = B // N_CORES
    in_maps = []
    for c in range(N_CORES):
        sl = slice(c * Bc, (c + 1) * Bc)
        in_maps.append({
            "comb": np.ascontiguousarray(y_pred_comb[sl]).reshape(NT, P, FREE),
            "idxg": (np.ascontiguousarray(y_comb[sl]).astype(np.float32)
                     + (1.0 - np.ascontiguousarray(y_stroke[sl])[..., 0]) * BIG
                     ).reshape(NT, P, Tp),
            "ps": np.ascontiguousarray(y_pred_stroke[sl]).reshape(NT, P, Tp),
            "ys": np.ascontiguousarray(y_stroke[sl]).reshape(NT, P, Tp),
        })
    return in_maps


def kernel(y_pred_stroke, y_pred_comb, y_stroke, y_comb):
    key = (NT, Tp)
    if key not in _NC_CACHE:
        _NC_CACHE[key] = _build(NT, Tp)
    nc = _NC_CACHE[key]
    in_maps = make_in_maps(y_pred_stroke, y_pred_comb, y_stroke, y_comb)
    res = run_bass_kernel_spmd(nc, in_maps, list(range(N_CORES)))
    total = 0.0
    for r in res.results:
        total += r["out"].astype(np.float64).sum()
    return np.asarray([total / (B * S)], dtype=np.float32)
